# revision 61
# baseline (speedup 1.0000x reference)
"""Trainium2 Bass kernel for nn_CrossAttention (B=2, SQ=SK=2048, D=256, H=8, HD=32, DO=256).

Sharding: 8 cores = 2 (batch) x 4 (head-pairs). Core c handles batch b=c//4 and
heads {2*(c%4), 2*(c%4)+1}. Each core produces a partial yT[o,k] (its two heads'
contribution to the output projection); the host sums the 4 partials per batch,
adds bo and applies leaky-relu while unsharding.

Math: softmax_k(-|q-k|^2) == softmax_k(2*q.k - |k|^2)  (the |q|^2 row term cancels).
The -|k|^2 row rides along the score matmul as an augmented K=33 contraction row.
Scores stay in fp32 PSUM; ScalarE evacuates them through exp() into bf16 E tiles
while accumulating the softmax denominators Z. attn^T = (v/Z)^T @ E via 4x
column-tiled PE matmuls accumulating over q in two PSUM banks.
"""

import os
import numpy as np

import concourse.bass as bass
import concourse.mybir as mybir
import concourse.tile as tile
from concourse import bacc

F16 = mybir.dt.float16
F32 = mybir.dt.float32
BF16 = mybir.dt.bfloat16

B, SQ, SK, D, H, HD, DO = 2, 2048, 2048, 256, 8, 32, 256
NQC = SQ // 128          # 16 q-chunks
NKB = SK // 512          # 4 k-blocks
N_CORES = 8

AF = mybir.ActivationFunctionType

# Z row-sum path:
#  "act"   = exp accum_out on ACT for all chunks (proven, ~190-280ns/op on ACT)
#  "mixed" = chunk-a summed via DVE tensor_reduce, chunk-b via ACT accum
#  "dve"   = E1+E2 on DVE (2x bf16 TT) then one DVE tensor_reduce per (h,qc);
#            ACT stays pure exp.
# NOTE: DVE ops with accum_out (tensor_scalar, tensor_tensor_reduce) crash the
# real HW with NRT_EXEC_UNIT_UNRECOVERABLE; plain tensor_reduce is fine.
Z_PATH = os.environ.get("KERNEL_Z_PATH", "mixed")
# In "mixed" mode: how many of the 64 chunk row-sums go to DVE tensor_reduce
# (the rest use ACT exp-accum).  GPSIMD cannot do free-axis reduces.
Z_DVE_N = int(os.environ.get("KERNEL_Z_DVE_N", "40"))


def build_module(debug: bool = False):
    nc = bacc.Bacc("TRN2", target_bir_lowering=False, debug=debug,
                   num_devices=N_CORES)

    # ---- DRAM I/O (per-core views; host feeds per-core slices) ----
    tT_d = nc.dram_tensor("tT", (2, 128, SQ), F16, kind="ExternalInput").ap()
    xT_d = nc.dram_tensor("xT", (2, 128, SK), F16, kind="ExternalInput").ap()
    wq_d = nc.dram_tensor("wq", (3, 128, 128), F16, kind="ExternalInput").ap()
    wk_d = nc.dram_tensor("wk", (3, 128, 128), F16, kind="ExternalInput").ap()
    wv_d = nc.dram_tensor("wv", (3, 128, 64), F16, kind="ExternalInput").ap()
    wo_d = nc.dram_tensor("wo", (2, 128, 128), F16, kind="ExternalInput").ap()
    c33_d = nc.dram_tensor("c33", (128, 33), F16, kind="ExternalInput").ap()
    yT_d = nc.dram_tensor("yT", (2, 128, SK), F32, kind="ExternalOutput").ap()

    with tile.TileContext(nc) as tc:
        with (
            tc.tile_pool(name="singles", bufs=1) as singles,
            tc.tile_pool(name="spsum", bufs=3, space="PSUM") as spool,
            tc.tile_pool(name="apsum", bufs=1, space="PSUM") as apool,
            tc.tile_pool(name="epool", bufs=8) as epool,
            tc.tile_pool(name="small", bufs=6) as small,
        ):
            # ---- static SBUF tiles + input DMA ----
            tT = singles.tile([128, 2, SQ], F16)
            xT = singles.tile([128, 2, SK], F16)
            wq = singles.tile([128, 3, 128], F16)
            wk = singles.tile([128, 3, 128], F16)
            wv = singles.tile([128, 3, 64], F16)
            wo = singles.tile([128, 2, 128], F16)
            c33 = singles.tile([128, 33], F16)
            ones = singles.tile([128, 512], F16)
            qT = singles.tile([128, SQ], F16)
            kT = singles.tile([128, SK], F16)
            kT2 = singles.tile([128, SK], F16)
            v_sb = singles.tile([128, NQC, 64], F32)
            attn_sb = singles.tile([128, 2 * 512], F16)
            yT_sb = singles.tile([128, 2, SK], F32)

            scratch = singles.tile([128, 1024], BF16)
            # preload the exp table set while input DMAs are in flight
            nc.vector.memset(ones[:, :], 1.0)
            nc.scalar.activation(scratch[0:1, 0:1], ones[0:1, 0:1], AF.Exp)
            # The first-exp critical path needs xT half 0 (both d-chunks) and
            # wk first: issue the two half-0 chunks on different engines at
            # t=0 so their transfers overlap, then the small weight DMAs.
            h0, h1 = slice(0, 1024), slice(1024, 2048)
            nc.sync.dma_start(out=xT[:, 0, h0], in_=xT_d[0][:, h0])
            nc.gpsimd.dma_start(out=xT[:, 1, h0], in_=xT_d[1][:, h0])
            for c in range(3):
                nc.sync.dma_start(out=wk[:, c, :], in_=wk_d[c])
                nc.gpsimd.dma_start(out=wq[:, c, :], in_=wq_d[c])
            nc.gpsimd.dma_start(out=c33[:, :], in_=c33_d[:, :])
            nc.sync.dma_start(out=xT[:, 0, h1], in_=xT_d[0][:, h1])
            nc.gpsimd.dma_start(out=xT[:, 1, h1], in_=xT_d[1][:, h1])
            for half in range(2):
                hs = slice(half * 1024, half * 1024 + 1024)
                for c in range(2):
                    (nc.sync if c == 0 else nc.gpsimd).dma_start(
                        out=tT[:, c, hs], in_=tT_d[c][:, hs])
            for c in range(3):
                nc.gpsimd.dma_start(out=wv[:, c, :], in_=wv_d[c])
            for c in range(2):
                nc.gpsimd.dma_start(out=wo[:, c, :], in_=wo_d[c])

            # ---- qT / kT projections (augmented, both heads) ----
            # out partitions: 0-31 h0 rows, 32 ones/k2 row, 64-95 h1, 96 h1 aug
            # half-major order so the main loop can start on half 0
            for half in range(2):
                for (wmat, dst, is_k) in ((wk, kT, True), (wq, qT, False)):
                    pt = spool.tile([128, 1024], F32, tag="spsum")
                    for nb in range(2):
                        ns = slice(half * 1024 + nb * 512, half * 1024 + nb * 512 + 512)
                        psl = slice(nb * 512, nb * 512 + 512)
                        src = xT if is_k else tT
                        for c in range(3):
                            lhsT = wmat[:, c, :] if c < 2 else wmat[0:1, 2, :]
                            rhs = src[:, c, ns] if c < 2 else ones[0:1, 0:512]
                            nc.tensor.matmul(pt[:, psl], lhsT, rhs,
                                             start=(c == 0), stop=(c == 2))
                    hsl = slice(half * 1024, half * 1024 + 1024)
                    if not is_k:
                        nc.vector.tensor_copy(dst[:, hsl], pt[:, :])
                    else:
                        # k rows: DVE evac fp16 + square; the c33 matmul then
                        # ACCUMULATES -0.25*sum_e (2k)^2 into the (stopped)
                        # projection tile's zero rows 32/96 (has_written bits
                        # persist, so start=False adds; the group check is a
                        # partition-insensitive sim artifact), and a second
                        # full DVE evac lands the k2 rows in fp16 kT.  This
                        # keeps ACT's in-order stream free of prologue copies.
                        nc.vector.tensor_copy(dst[:, hsl], pt[:, :])
                        nc.scalar.activation(kT2[:, hsl], pt[:, :], AF.Square)
                        for hh in range(2):
                            base = 64 * hh
                            for nb in range(2):
                                psl = slice(nb * 512, nb * 512 + 512)
                                ns = slice(half * 1024 + nb * 512,
                                           half * 1024 + nb * 512 + 512)
                                nc.tensor.matmul(
                                    pt[base:base + 33, psl],
                                    c33[base:base + 32, :],
                                    kT2[base:base + 32, ns],
                                    start=False, stop=True,
                                    skip_group_check=True,
                                )
                        nc.vector.tensor_copy(dst[:, hsl], pt[:, :])

            # ---- v projection: v[qc] = t_local @ Wv (+bv), out [128q, 64] ----
            # vt rotates through the (not yet active) attnA slot so the score
            # pool is free for the main loop from the first q-chunk.
            for qc in range(NQC):
                vt = apool.tile([128, 64], F32, tag="attnA")
                qs = slice(qc * 128, (qc + 1) * 128)
                for c in range(3):
                    lhsT = tT[:, c, qs] if c < 2 else ones[0:1, 0:128]
                    rhs = wv[:, c, :] if c < 2 else wv[0:1, 2, :]
                    nc.tensor.matmul(vt[:, :], lhsT, rhs,
                                     start=(c == 0), stop=(c == 2))
                nc.vector.tensor_copy(v_sb[:, qc, :], vt[:, :])

            # ---- attention accumulators (live across the whole q loop) ----
            attnA = apool.tile([128, 512], F32, tag="attnA")
            attnB = apool.tile([128, 512], F32, tag="attnB")

            # ---- main loop over q-chunks and heads ----
            for qc in range(NQC):
                qs = slice(qc * 128, (qc + 1) * 128)
                for h in range(2):
                    base = 64 * h
                    lhs = qT[base:base + 33, qs]
                    e_tiles = []
                    zp = small.tile([128, 2], F32, tag="zp")
                    for half in range(2):
                        sp = spool.tile([128, 1024], F32, tag="spsum")
                        for nb in range(2):
                            ks = slice(half * 1024 + nb * 512,
                                       half * 1024 + nb * 512 + 512)
                            nc.tensor.matmul(sp[:, nb * 512:(nb + 1) * 512],
                                             lhs, kT[base:base + 33, ks],
                                             start=True, stop=True)
                        et = epool.tile([128, 1024], BF16, tag="E")
                        slot = (qc * 2 + h) * 2 + half
                        on_dve = (slot * Z_DVE_N) % 64 + Z_DVE_N >= 64
                        if qc == NQC - 1:
                            # tail: keep the last chunks' Z off the DVE
                            # critical chain (ACT is idle after its last exp)
                            on_dve = False
                        if Z_PATH == "act" or not on_dve:
                            nc.scalar.activation(et[:, :], sp[:, :], AF.Exp,
                                                 accum_out=zp[:, half:half + 1])
                        else:
                            nc.scalar.activation(et[:, :], sp[:, :], AF.Exp)
                            nc.vector.tensor_reduce(
                                out=zp[:, half:half + 1], in_=et[:, :],
                                op=mybir.AluOpType.add,
                                axis=mybir.AxisListType.X)
                        e_tiles.append(et)
                    zr = small.tile([128, 1], F32, tag="zr")
                    nc.vector.tensor_add(zr[:, :], zp[:, 0:1], zp[:, 1:2])
                    nc.vector.reciprocal(zr[:, :], zr[:, :])
                    vp = small.tile([128, 32], BF16, tag="vp")
                    nc.vector.tensor_scalar_mul(vp[:, :],
                                                v_sb[:, qc, 32 * h:32 * h + 32],
                                                zr[:, 0:1])
                    for kb in range(NKB):
                        acc = attnA if kb < 2 else attnB
                        col = 64 * (kb % 2) + 32 * h
                        esrc = e_tiles[kb // 2]
                        nc.tensor.matmul(
                            acc[col:col + 32, :], vp[:, :],
                            esrc[:, (kb % 2) * 512:(kb % 2) * 512 + 512],
                            start=(qc == 0), stop=(qc == NQC - 1),
                            tile_position=(0, col), skip_group_check=True,
                        )

            # ---- evac attn, output projection (ACT is idle; split with DVE) ----
            nc.vector.tensor_copy(attn_sb[:, 0:512], attnA[:, :])
            nc.scalar.activation(attn_sb[:, 512:1024], attnB[:, :], AF.Copy)
            for kb in range(NKB):
                rbase = 64 * (kb % 2)
                rsl = slice((kb // 2) * 512, (kb // 2) * 512 + 512)
                ysl = slice(kb * 512, (kb + 1) * 512)
                yp = spool.tile([128, 1024], F32, tag="spsum")
                for ob in range(2):
                    nc.tensor.matmul(yp[:, ob * 512:(ob + 1) * 512],
                                     wo[rbase:rbase + 64, ob, :],
                                     attn_sb[rbase:rbase + 64, rsl],
                                     start=True, stop=True)
                # one double-width evac per kb, alternating DVE/ACT
                src = yp[:, :].rearrange("p (o n) -> p o n", o=2)
                if kb % 2 == 0:
                    nc.vector.tensor_copy(yT_sb[:, :, ysl], src)
                else:
                    nc.scalar.activation(yT_sb[:, :, ysl], src, AF.Copy)
                for ob in range(2):
                    eng = nc.sync if ob == 0 else nc.gpsimd
                    eng.dma_start(out=yT_d[ob][:, ysl], in_=yT_sb[:, ob, ysl])

    nc.compile()
    return nc


# ------------------------- host side -------------------------

def _prep_core_inputs(c, x, t_local, Wk, bk, Wq, bq, Wv, bv, Wo, bo):
    b = c // 4
    h0 = 2 * (c % 4)
    heads = (h0, h0 + 1)

    tT = np.ascontiguousarray(t_local[b].T).astype(np.float16).reshape(2, 128, SQ)
    xT = np.ascontiguousarray(x[b].T).astype(np.float16).reshape(2, 128, SK)

    wq = np.zeros((384, 128), np.float32)
    wk = np.zeros((384, 128), np.float32)
    wv = np.zeros((384, 64), np.float32)
    for i, h in enumerate(heads):
        c0 = 64 * i
        wq[:D, c0:c0 + 32] = Wq[:, h, :]
        wq[D, c0:c0 + 32] = bq[h]
        wq[D, c0 + 32] = 1.0
        wk[:D, c0:c0 + 32] = 2.0 * Wk[:, h, :]
        wk[D, c0:c0 + 32] = 2.0 * bk[h]
        wv[:D, 32 * i:32 * i + 32] = Wv[:, h, :]
        wv[D, 32 * i:32 * i + 32] = bv[h]

    wo = np.zeros((2, 128, 128), np.float32)
    for ob in range(2):
        for i, h in enumerate(heads):
            blk = Wo[h, :, 128 * ob:128 * ob + 128]   # [HD, 128]
            wo[ob, 32 * i:32 * i + 32, :] = blk
            wo[ob, 64 + 32 * i:64 + 32 * i + 32, :] = blk

    c33 = np.zeros((128, 33), np.float32)
    c33[0:32, 32] = -0.25
    c33[64:96, 32] = -0.25

    return {
        "tT": tT, "xT": xT,
        "wq": wq.astype(np.float16).reshape(3, 128, 128),
        "wk": wk.astype(np.float16).reshape(3, 128, 128),
        "wv": wv.astype(np.float16).reshape(3, 128, 64),
        "wo": wo.astype(np.float16),
        "c33": c33.astype(np.float16),
    }


_NC_CACHE = {}


def kernel(x, t_local, Wk, bk, Wq, bq, Wv, bv, Wo, bo):
    from concourse import bass_utils

    args = [np.asarray(np.asarray(a), np.float32) for a in
            (x, t_local, Wk, bk, Wq, bq, Wv, bv, Wo, bo)]
    x, t_local, Wk, bk, Wq, bq, Wv, bv, Wo, bo = args

    if "nc" not in _NC_CACHE:
        _NC_CACHE["nc"] = build_module()
    nc = _NC_CACHE["nc"]

    in_maps = [_prep_core_inputs(c, *args) for c in range(N_CORES)]
    # The axon tunnel very occasionally reports a transient
    # NRT_EXEC_UNIT_UNRECOVERABLE on a first run; retry before giving up.
    import time as _time
    for attempt in range(3):
        try:
            res = bass_utils.run_bass_kernel_spmd(
                nc, in_maps, core_ids=list(range(N_CORES)),
                trace=bool(int(os.environ.get("KERNEL_TRACE", "0") or 0)),
            )
            break
        except Exception:
            if attempt == 2:
                raise
            _time.sleep(5.0)
    _NC_CACHE["last_results"] = res

    y = np.zeros((B, SK, DO), np.float32)
    for b in range(B):
        acc = np.zeros((2, 128, SK), np.float32)
        for c in range(4 * b, 4 * b + 4):
            acc += res.results[c]["yT"].astype(np.float32)
        y[b] = acc.reshape(DO, SK).T
    y += bo[None, None, :]
    return np.where(y >= 0, y, 0.01 * y).astype(np.float32)
